# revision 7
# baseline (speedup 1.0000x reference)
"""Row-normalize kernel for nn_EstimateAdj (N=8192) on 8 trn2 NeuronCores.

Math (per reference):
    mx     = estimated_adj * ori + I
    rowsum = mx.sum(axis=1)
    out    = (1/rowsum)[:, None] * mx

Sharding: 1D row partition across 8 cores (1024 rows each). Row-sum,
reciprocal and row-scale are row-local, so the device program is uniform
across cores. The identity matrix is handled without any core-dependent
addressing:
  - its contribution to rowsum is the reduction's initial value (1.0)
  - its contribution to the output (out[i,i] += r_inv[i]) is an O(N)
    host-side fix-up using the r_inv values computed on device.

Per core: 8 row-tiles of [128, 8192] f32. Per tile:
  DMA in est/ori -> DVE tensor_tensor_reduce (mx = est*ori, rowsum with
  init 1.0) -> DVE reciprocal -> ScalarE copy-with-scale (out = mx * r_inv)
  -> DMA out. Memory-bound: 96 MB HBM traffic per core.
"""

import numpy as np

import concourse.bacc as bacc
import concourse.bass as bass
import concourse.tile as tile
from concourse import mybir
from concourse.bass_utils import run_bass_kernel_spmd

N = 8192
N_CORES = 8
ROWS = N // N_CORES  # rows per core
P = 128              # SBUF partitions
TILES = ROWS // P    # row-tiles per core

_NC_CACHE: dict = {}


def _build_nc(repeats: int = 1) -> bass.Bass:
    """Build the per-core program. repeats>1 wraps the whole body in a
    hardware loop that redoes identical work — used only for timing."""
    nc = bacc.Bacc(None)
    est = nc.dram_tensor("est", [ROWS, N], mybir.dt.float32, kind="ExternalInput")
    ori = nc.dram_tensor("ori", [ROWS, N], mybir.dt.float32, kind="ExternalInput")
    out = nc.dram_tensor("out", [ROWS, N], mybir.dt.float32, kind="ExternalOutput")
    rinv = nc.dram_tensor("rinv", [TILES, P], mybir.dt.float32, kind="ExternalOutput")

    from contextlib import ExitStack, nullcontext

    with tile.TileContext(nc) as tc, ExitStack() as ctx:
        est_pool = ctx.enter_context(tc.tile_pool(name="est_pool", bufs=3))
        ori_pool = ctx.enter_context(tc.tile_pool(name="ori_pool", bufs=2))
        small = ctx.enter_context(tc.tile_pool(name="small", bufs=4))
        with tc.For_i(0, repeats, 1) if repeats > 1 else nullcontext():
            for t in range(TILES):
                est_t = est_pool.tile([P, N], mybir.dt.float32)
                ori_t = ori_pool.tile([P, N], mybir.dt.float32)
                nc.sync.dma_start(out=est_t[:], in_=est[t * P : (t + 1) * P, :])
                nc.sync.dma_start(out=ori_t[:], in_=ori[t * P : (t + 1) * P, :])

                rowsum = small.tile([P, 1], mybir.dt.float32)
                rinv_t = small.tile([P, 1], mybir.dt.float32)
                # mx = (est*1.0)*ori in-place into est_t; rowsum = sum(mx row)
                nc.vector.scalar_tensor_tensor(
                    out=est_t[:],
                    in0=est_t[:],
                    scalar=1.0,
                    in1=ori_t[:],
                    op0=mybir.AluOpType.mult,
                    op1=mybir.AluOpType.mult,
                    accum_out=rowsum[:],
                )
                # +1.0 accounts for the identity's diagonal in this row
                nc.vector.tensor_scalar_add(rowsum[:], rowsum[:], 1.0)
                nc.vector.reciprocal(out=rinv_t[:], in_=rowsum[:])
                # out = mx * r_inv on ScalarE (per-partition scale), into ori_t
                nc.scalar.mul(out=ori_t[:], in_=est_t[:], mul=rinv_t[:])
                nc.sync.dma_start(out=out[t * P : (t + 1) * P, :], in_=ori_t[:])
                nc.sync.dma_start(out=rinv[t : t + 1, :].rearrange("a b -> b a"), in_=rinv_t[:])
    nc.finalize()
    return nc


def _get_nc(repeats: int = 1) -> bass.Bass:
    if repeats not in _NC_CACHE:
        _NC_CACHE[repeats] = _build_nc(repeats)
    return _NC_CACHE[repeats]


def run_sharded(estimated_adj: np.ndarray, ori: np.ndarray, repeats: int = 1, **run_kwargs):
    """Shard inputs, run the SPMD kernel on 8 cores, return BassKernelResults."""
    est = np.ascontiguousarray(np.asarray(estimated_adj, dtype=np.float32))
    orig = np.ascontiguousarray(np.asarray(ori, dtype=np.float32))
    in_maps = [
        {
            "est": est[c * ROWS : (c + 1) * ROWS],
            "ori": orig[c * ROWS : (c + 1) * ROWS],
        }
        for c in range(N_CORES)
    ]
    return run_bass_kernel_spmd(_get_nc(repeats), in_maps, list(range(N_CORES)), **run_kwargs)


def assemble(results) -> np.ndarray:
    """Gather per-core outputs into the full [N, N] result (with diag fix)."""
    out = np.concatenate([r["out"] for r in results], axis=0)
    # rinv[t, p] = 1/rowsum of local row t*128+p
    rinv = np.concatenate([np.asarray(r["rinv"]).reshape(-1) for r in results])
    idx = np.arange(N)
    out[idx, idx] += rinv
    return out


def kernel(estimated_adj: np.ndarray, ori: np.ndarray) -> np.ndarray:
    res = run_sharded(estimated_adj, ori)
    return assemble(res.results)


# revision 11
# speedup vs baseline: 3.7267x; 3.7267x over previous
"""Row-normalize kernel for nn_EstimateAdj (N=8192) on 8 trn2 NeuronCores.

Math (per reference):
    mx     = estimated_adj * ori + I
    rowsum = mx.sum(axis=1)
    out    = (1/rowsum)[:, None] * mx

Sharding: 1D row partition across 8 cores (1024 rows each). Row-sum,
reciprocal and row-scale are row-local, so the device program is uniform
across cores. The identity matrix is handled without any core-dependent
addressing:
  - its contribution to rowsum is the reduction's initial value (1.0)
  - its contribution to the output (out[i,i] += r_inv[i]) is an O(N)
    host-side fix-up using the r_inv values computed on device.

Per core: 8 row-tiles of [128, 8192] f32. Per tile:
  DMA in est/ori -> DVE tensor_tensor_reduce (mx = est*ori, rowsum with
  init 1.0) -> DVE reciprocal -> ScalarE copy-with-scale (out = mx * r_inv)
  -> DMA out. Memory-bound: 96 MB HBM traffic per core.
"""

import numpy as np

import concourse.bacc as bacc
import concourse.bass as bass
import concourse.tile as tile
from concourse import mybir
from concourse.bass_utils import run_bass_kernel_spmd

N = 8192
N_CORES = 8
ROWS = N // N_CORES  # rows per core
P = 128              # SBUF partitions
TILES = ROWS // P    # row-tiles per core

_NC_CACHE: dict = {}


def _build_nc(repeats: int = 1) -> bass.Bass:
    """Build the per-core program. repeats>1 wraps the whole body in a
    hardware loop that redoes identical work — used only for timing."""
    nc = bacc.Bacc(None)
    est = nc.dram_tensor("est", [ROWS, N], mybir.dt.float32, kind="ExternalInput")
    ori = nc.dram_tensor("ori", [ROWS, N], mybir.dt.float32, kind="ExternalInput")
    out = nc.dram_tensor("out", [ROWS, N], mybir.dt.float32, kind="ExternalOutput")
    # [P, TILES]: rinv[p, t] = 1/rowsum of local row t*P+p (host transposes)
    rinv = nc.dram_tensor("rinv", [P, TILES], mybir.dt.float32, kind="ExternalOutput")

    from contextlib import ExitStack, nullcontext

    with tile.TileContext(nc) as tc, ExitStack() as ctx:
        est_pool = ctx.enter_context(tc.tile_pool(name="est_pool", bufs=3))
        ori_pool = ctx.enter_context(tc.tile_pool(name="ori_pool", bufs=2))
        small = ctx.enter_context(tc.tile_pool(name="small", bufs=4))
        singles = ctx.enter_context(tc.tile_pool(name="singles", bufs=1))
        with tc.For_i(0, repeats, 1) if repeats > 1 else nullcontext():
            # r_inv for all tiles, written column t per tile, one store at end
            rinv_all = singles.tile([P, TILES], mybir.dt.float32)
            for t in range(TILES):
                est_t = est_pool.tile([P, N], mybir.dt.float32)
                ori_t = ori_pool.tile([P, N], mybir.dt.float32)
                # loads on the SP HWDGE ring only — stores go via ACT so a
                # store's compute-wait never blocks load issue
                nc.sync.dma_start(out=est_t[:], in_=est[t * P : (t + 1) * P, :])
                nc.sync.dma_start(out=ori_t[:], in_=ori[t * P : (t + 1) * P, :])

                rowsum = small.tile([P, 1], mybir.dt.float32)
                # mx = (est*1.0)*ori in-place into est_t; rowsum = sum(mx row)
                nc.vector.scalar_tensor_tensor(
                    out=est_t[:],
                    in0=est_t[:],
                    scalar=1.0,
                    in1=ori_t[:],
                    op0=mybir.AluOpType.mult,
                    op1=mybir.AluOpType.mult,
                    accum_out=rowsum[:],
                )
                # +1.0 accounts for the identity's diagonal in this row
                nc.vector.tensor_scalar_add(rowsum[:], rowsum[:], 1.0)
                nc.vector.reciprocal(out=rinv_all[:, t : t + 1], in_=rowsum[:])
                # out = mx * r_inv on ScalarE (per-partition scale), into ori_t
                nc.scalar.mul(out=ori_t[:], in_=est_t[:], mul=rinv_all[:, t : t + 1])
                nc.scalar.dma_start(out=out[t * P : (t + 1) * P, :], in_=ori_t[:])
            nc.scalar.dma_start(out=rinv[:, :], in_=rinv_all[:])
    nc.finalize()
    return nc


def _get_nc(repeats: int = 1) -> bass.Bass:
    if repeats not in _NC_CACHE:
        _NC_CACHE[repeats] = _build_nc(repeats)
    return _NC_CACHE[repeats]


def run_sharded(estimated_adj: np.ndarray, ori: np.ndarray, repeats: int = 1, **run_kwargs):
    """Shard inputs, run the SPMD kernel on 8 cores, return BassKernelResults."""
    est = np.ascontiguousarray(np.asarray(estimated_adj, dtype=np.float32))
    orig = np.ascontiguousarray(np.asarray(ori, dtype=np.float32))
    in_maps = [
        {
            "est": est[c * ROWS : (c + 1) * ROWS],
            "ori": orig[c * ROWS : (c + 1) * ROWS],
        }
        for c in range(N_CORES)
    ]
    return run_bass_kernel_spmd(_get_nc(repeats), in_maps, list(range(N_CORES)), **run_kwargs)


def assemble(results) -> np.ndarray:
    """Gather per-core outputs into the full [N, N] result (with diag fix)."""
    out = np.concatenate([r["out"] for r in results], axis=0)
    # rinv[p, t] = 1/rowsum of local row t*128+p -> transpose to row order
    rinv = np.concatenate([np.asarray(r["rinv"]).T.reshape(-1) for r in results])
    idx = np.arange(N)
    out[idx, idx] += rinv
    return out


def kernel(estimated_adj: np.ndarray, ori: np.ndarray) -> np.ndarray:
    res = run_sharded(estimated_adj, ori)
    return assemble(res.results)
